# revision 24
# baseline (speedup 1.0000x reference)
"""Trainium2 Bass kernel for nn_CategoricalEmbedding (embedding_lookup).

Problem: 32 categorical variables; per variable i:
    e = emb_tables[i][x[:, i]]          # (B, d_i) gather
    y = e @ proj_w[i].T + proj_b[i]     # (B, 512)
    out[i] = LayerNorm(y) * ln_g[i] + ln_b[i]
stacked to (32, B, 512), B = 8192.

Key algebraic insight: indices are bounded (< 1000 per the problem spec), and
every post-gather op (projection, LayerNorm, scale/shift) is a pure row-wise
function of the gathered row.  So the whole chain per variable collapses to a
lookup into a precomputed table
    T_i[k] = LayerNorm(emb_i[k] @ W_i^T + pb_i) * g_i + beta_i   (k < 1024)
and out[i, b] = T_i[x[b, i]].

Sharding: expert-style, 4 variables per core across 8 cores (CARDS repeats
with period 4, so every core sees the same shapes -> single SPMD NEFF).

Device kernel per core:
  phase 1 (compute, tiny): build T_v (1024, 512) on device via PE matmul
          (with PE transposes for the lhsT/rhs layouts) + bn_stats LayerNorm.
  phase 2 (memory-bound): indirect-DMA row gather of T_v rows per batch
          element, then contiguous store to the output slab.
"""

import math
import os

import numpy as np

import concourse.bacc as bacc
import concourse.bass as bass
import concourse.mybir as mybir
import concourse.tile as tile
from concourse.bass_utils import run_bass_kernel_spmd
from concourse.masks import make_identity

# ---------------------------------------------------------------- constants
N_VARS = 32
N_CORES = 8
VPC = N_VARS // N_CORES  # vars per core = 4
B = 8192
D = 512
TR = 1024  # table rows kept on device (indices are < 1000 per spec)
EPS = 1e-5
CARDS = [1000, 5000, 20000, 50000] * 8
DIMS = [int(math.ceil(c**0.5) + 1) for c in CARDS]  # [33, 72, 143, 225] * 8
DIMS_LOCAL = DIMS[:VPC]  # same for every core

F32 = mybir.dt.float32
I16 = mybir.dt.int16

# gather chunking: 4 chunks of 2048 rows per variable
G_CHUNK_ROWS = 2048
G_CHUNKS = B // G_CHUNK_ROWS  # 4
G_FREE = G_CHUNK_ROWS // 128  # 16 rows per partition per chunk
G_SLOTS = G_CHUNK_ROWS // 16  # 128 int16 idx slots per 16-partition group


def _kchunks(d):
    """Split contraction dim d into <=128-partition chunks."""
    return [(k0, min(k0 + 128, d)) for k0 in range(0, d, 128)]


# ---------------------------------------------------------------- device IR
def _build_nc(repeat=1, phase="full", lean_ln=False):
    """repeat>1 duplicates the whole compute+gather body (same data) for
    dispatch-overhead-free wall-clock benchmarking via (T_k - T_1)/(k-1).
    phase: "full" | "pre" (no gather/store) | "pregather" (no output store),
    for component timing only (non-"full" variants produce wrong output).
    lean_ln=True compiles the specialization for proj_b==0, ln_g==1, ln_b==0
    (validated host-side before selecting this variant)."""
    nc = bacc.Bacc("TRN2", target_bir_lowering=False, debug=False)

    emb_in = [nc.dram_tensor(f"emb{v}", (TR, DIMS_LOCAL[v]), F32, kind="ExternalInput")
              for v in range(VPC)]
    w_in = [nc.dram_tensor(f"w{v}", (D, DIMS_LOCAL[v]), F32, kind="ExternalInput")
            for v in range(VPC)]
    # aux[v, 0] = proj_b, aux[v, 1] = ln_g, aux[v, 2] = ln_b
    aux_in = nc.dram_tensor("aux", (VPC, 3, D), F32, kind="ExternalInput")
    # dma_gather int16 indices: per (var, chunk), idx j -> partition j%16,
    # slot j//16, replicated across the 8 Q7-core 16-partition groups
    idx_in = nc.dram_tensor("idx", (128, VPC * G_CHUNKS * G_SLOTS), I16,
                            kind="ExternalInput")
    y_out = nc.dram_tensor("y", (VPC * B, D), F32, kind="ExternalOutput")

    # per-variable precomputed tables (device-internal)
    t_dram = [nc.dram_tensor(f"t{v}", (TR, D), F32, kind="Internal")
              for v in range(VPC)]

    with tile.TileContext(nc) as tc:
        with (
            tc.tile_pool(name="consts", bufs=1) as consts,
            tc.tile_pool(name="wpool", bufs=2) as wpool,
            tc.tile_pool(name="lpool", bufs=3) as lpool,
            tc.tile_pool(name="tpose", bufs=2, space="PSUM") as tpose_psum,
            tc.tile_pool(name="mmps", bufs=2, space="PSUM") as mm_psum,
            tc.tile_pool(name="tpool", bufs=3) as tpool,
            tc.tile_pool(name="stat", bufs=4) as stat,
            tc.tile_pool(name="gpool", bufs=4) as gpool,
        ):
            identity = consts.tile([128, 128], F32)
            make_identity(nc, identity[:])
            eps_t = consts.tile([128, 1], F32)
            nc.vector.memset(eps_t[:], EPS)
            idx_sb = consts.tile([128, VPC * G_CHUNKS * G_SLOTS], I16)
            nc.sync.dma_start(out=idx_sb[:], in_=idx_in[:, :])

            for rep in range(repeat):
              for v in range(VPC):
                d = DIMS_LOCAL[v]
                kcs = _kchunks(d)

                # broadcast aux rows across partitions (DRE replication DMA)
                pb_b = g_b = bb_b = None
                if not lean_ln:
                    aux_t = []
                    for a in range(3):
                        t = wpool.tile([128, D], F32, tag=f"aux{a}",
                                       name=f"aux_v{v}_{a}")
                        src = aux_in[v, a, :]
                        bcast = bass.AP(tensor=src.tensor, offset=src.offset,
                                        ap=[[0, 128]] + src.ap)
                        nc.gpsimd.dma_start(out=t[:], in_=bcast)
                        aux_t.append(t)
                    pb_b, g_b, bb_b = aux_t

                # rhs = W^T  (d, 512) assembled via PE transposes of W tiles
                rhs = [wpool.tile([128, D], F32, tag=f"rhs{kci}",
                                  name=f"rhs_v{v}_{kci}")
                       for kci in range(len(kcs))]
                for t4 in range(D // 128):
                    w_t = wpool.tile([128, d], F32, tag="wstage")
                    nc.sync.dma_start(out=w_t[:],
                                      in_=w_in[v][t4 * 128:(t4 + 1) * 128, :])
                    for kci, (k0, k1) in enumerate(kcs):
                        kk = k1 - k0
                        tp = tpose_psum.tile([128, 128], F32)
                        nc.tensor.transpose(out=tp[:kk, :], in_=w_t[:, k0:k1],
                                            identity=identity[:])
                        nc.vector.tensor_copy(
                            out=rhs[kci][:kk, t4 * 128:(t4 + 1) * 128],
                            in_=tp[:kk, :])

                # per 128-row tile of the table: matmul + LayerNorm
                for s in range(TR // 128):
                    e_t = lpool.tile([128, d], F32, tag="emb")
                    nc.sync.dma_start(out=e_t[:],
                                      in_=emb_in[v][s * 128:(s + 1) * 128, :])
                    lts = []
                    for kci, (k0, k1) in enumerate(kcs):
                        kk = k1 - k0
                        tp = tpose_psum.tile([128, 128], F32)
                        nc.tensor.transpose(out=tp[:kk, :], in_=e_t[:, k0:k1],
                                            identity=identity[:])
                        lt = lpool.tile([128, 128], F32, tag=f"lhsT{kci}")
                        nc.vector.tensor_copy(out=lt[:kk, :], in_=tp[:kk, :])
                        lts.append(lt)

                    mm = mm_psum.tile([128, D], F32)
                    for kci, (k0, k1) in enumerate(kcs):
                        kk = k1 - k0
                        nc.tensor.matmul(
                            out=mm[:, :],
                            lhsT=lts[kci][:kk, :],
                            rhs=rhs[kci][:kk, :],
                            start=(kci == 0),
                            stop=(kci == len(kcs) - 1),
                        )

                    y_t = tpool.tile([128, D], F32, tag="yt")
                    stats = stat.tile([128, 6], F32, tag="bn")
                    mv = stat.tile([128, 2], F32, tag="mv")
                    if lean_ln:
                        # proj_b==0, g==1, beta==0: stats straight off PSUM,
                        # single normalize pass PSUM->SBUF
                        nc.vector.bn_stats(out=stats[:], in_=mm[:, :])
                    else:
                        # y = mm + proj_b  (reads PSUM, writes SBUF)
                        nc.vector.tensor_add(out=y_t[:], in0=mm[:, :],
                                             in1=pb_b[:])
                        nc.vector.bn_stats(out=stats[:], in_=y_t[:])
                    nc.vector.bn_aggr(out=mv[:], in_=stats[:])
                    # mv[:,1] <- 1/sqrt(var + eps)
                    nc.scalar.activation(
                        out=mv[:, 1:2], in_=mv[:, 1:2],
                        func=mybir.ActivationFunctionType.Sqrt,
                        bias=eps_t[:], scale=1.0, alpha=0.0)
                    nc.vector.reciprocal(out=mv[:, 1:2], in_=mv[:, 1:2])
                    # y = (y - mean) * rstd
                    nc.vector.tensor_scalar(
                        out=y_t[:], in0=mm[:, :] if lean_ln else y_t[:],
                        scalar1=mv[:, 0:1], scalar2=mv[:, 1:2],
                        op0=mybir.AluOpType.subtract,
                        op1=mybir.AluOpType.mult)
                    if not lean_ln:
                        # y = y * g + beta
                        nc.vector.tensor_mul(out=y_t[:], in0=y_t[:],
                                             in1=g_b[:])
                        nc.vector.tensor_add(out=y_t[:], in0=y_t[:],
                                             in1=bb_b[:])
                    nc.sync.dma_start(out=t_dram[v][s * 128:(s + 1) * 128, :],
                                      in_=y_t[:])

                # gather phase: T_v[x[b]] -> output slab rows [v*B, (v+1)*B)
                if phase == "pre":
                    continue
                for ci in range(G_CHUNKS):
                    gt = gpool.tile([128, G_FREE, D], F32, tag="gather")
                    off0 = (v * G_CHUNKS + ci) * G_SLOTS
                    nc.gpsimd.dma_gather(
                        out_ap=gt[:, :, :],
                        in_ap=t_dram[v][:, :],
                        idxs_ap=idx_sb[:, off0:off0 + G_SLOTS],
                        num_idxs=G_CHUNK_ROWS,
                        num_idxs_reg=G_CHUNK_ROWS,
                        elem_size=D,
                        single_packet=False,
                    )
                    if phase == "pregather":
                        continue
                    # gathered row j lands at (p=j%128, c=j//128)
                    row0 = v * B + ci * G_CHUNK_ROWS
                    y_view = y_out[row0:row0 + G_CHUNK_ROWS, :].rearrange(
                        "(c p) d -> p c d", p=128)
                    # alternate the two HWDGE rings (SP / ACT) for store issue
                    eng = nc.sync if ci % 2 == 0 else nc.scalar
                    eng.dma_start(out=y_view, in_=gt[:, :, :])

    nc.compile()
    return nc


_NC_CACHE = {}


def _get_nc(lean_ln=False):
    key = ("nc", lean_ln)
    if key not in _NC_CACHE:
        _NC_CACHE[key] = _build_nc(lean_ln=lean_ln)
    return _NC_CACHE[key]


# ---------------------------------------------------------------- host side
def _numpy_fallback(emb_tables, proj_w, proj_b, ln_g, ln_b, x):
    outs = []
    for i in range(len(emb_tables)):
        e = np.asarray(emb_tables[i], np.float32)[x[:, i]]
        y = e @ np.asarray(proj_w[i], np.float32).T + np.asarray(
            proj_b[i], np.float32)
        mu = y.mean(-1, keepdims=True, dtype=np.float32)
        var = np.square(y - mu).mean(-1, keepdims=True, dtype=np.float32)
        y = (y - mu) / np.sqrt(var + EPS)
        outs.append(y * np.asarray(ln_g[i], np.float32)
                    + np.asarray(ln_b[i], np.float32))
    return np.stack(outs).astype(np.float32)


def _prep_in_maps(emb_tables, proj_w, proj_b, ln_g, ln_b, x):
    in_maps = []
    for c in range(N_CORES):
        m = {}
        for v in range(VPC):
            gv = c * VPC + v
            d = DIMS[gv]
            et = np.asarray(emb_tables[gv], np.float32)
            if et.shape[0] >= TR:
                e = np.ascontiguousarray(et[:TR])
            else:
                e = np.zeros((TR, d), np.float32)
                e[:et.shape[0]] = et
            m[f"emb{v}"] = e
            m[f"w{v}"] = np.ascontiguousarray(np.asarray(proj_w[gv], np.float32))
        aux = np.empty((VPC, 3, D), np.float32)
        for v in range(VPC):
            gv = c * VPC + v
            aux[v, 0] = np.asarray(proj_b[gv], np.float32)
            aux[v, 1] = np.asarray(ln_g[gv], np.float32)
            aux[v, 2] = np.asarray(ln_b[gv], np.float32)
        m["aux"] = aux
        idx = np.empty((16, VPC, G_CHUNKS, G_SLOTS), np.int16)
        for v in range(VPC):
            gv = c * VPC + v
            xv = x[:, gv].astype(np.int16)
            # chunk ci, idx j -> partition j%16, slot j//16
            idx[:, v, :, :] = xv.reshape(G_CHUNKS, G_SLOTS, 16).transpose(
                2, 0, 1)
        idx = np.tile(idx.reshape(16, VPC * G_CHUNKS * G_SLOTS), (8, 1))
        m["idx"] = np.ascontiguousarray(idx)
        in_maps.append(m)
    return in_maps


def _run(emb_tables, proj_w, proj_b, ln_g, ln_b, x, trace=False):
    """Returns (out, exec_time_ns or None)."""
    x = np.asarray(x)
    ok = (
        len(emb_tables) == N_VARS
        and x.shape == (B, N_VARS)
        and all(np.asarray(emb_tables[i]).shape == (CARDS[i], DIMS[i])
                for i in range(N_VARS))
        and all(np.asarray(proj_w[i]).shape == (D, DIMS[i])
                for i in range(N_VARS))
        and int(x.min()) >= 0
        and int(x.max()) < TR
    )
    if not ok:
        return _numpy_fallback(emb_tables, proj_w, proj_b, ln_g, ln_b, x), None

    in_maps = _prep_in_maps(emb_tables, proj_w, proj_b, ln_g, ln_b, x)
    lean = (not np.any(np.asarray(proj_b))
            and np.all(np.asarray(ln_g) == 1.0)
            and not np.any(np.asarray(ln_b)))
    nc = _get_nc(lean_ln=lean)
    kw = {}
    if trace:
        kw = dict(trace=True, trace_cores=list(range(N_CORES)))
    res = run_bass_kernel_spmd(nc, in_maps, core_ids=list(range(N_CORES)), **kw)
    out = np.concatenate(
        [r["y"].reshape(VPC, B, D) for r in res.results], axis=0)
    return out, res.exec_time_ns


def kernel(emb_tables, proj_w, proj_b, ln_g, ln_b, x):
    out, _ = _run(emb_tables, proj_w, proj_b, ln_g, ln_b, x,
                  trace=bool(int(os.environ.get("KERNEL_TRACE", "0"))))
    return out


# revision 34
# speedup vs baseline: 1.0364x; 1.0364x over previous
"""Trainium2 Bass kernel for nn_CategoricalEmbedding (embedding_lookup).

Problem: 32 categorical variables; per variable i:
    e = emb_tables[i][x[:, i]]          # (B, d_i) gather
    y = e @ proj_w[i].T + proj_b[i]     # (B, 512)
    out[i] = LayerNorm(y) * ln_g[i] + ln_b[i]
stacked to (32, B, 512), B = 8192.

Key algebraic insight: indices are bounded (< 1000 per the problem spec), and
every post-gather op (projection, LayerNorm, scale/shift) is a pure row-wise
function of the gathered row.  So the whole chain per variable collapses to a
lookup into a precomputed table
    T_i[k] = LayerNorm(emb_i[k] @ W_i^T + pb_i) * g_i + beta_i   (k < 1024)
and out[i, b] = T_i[x[b, i]].

Sharding: expert-style, 4 variables per core across 8 cores (CARDS repeats
with period 4, so every core sees the same shapes -> single SPMD NEFF).

Device kernel per core:
  phase 1 (compute, tiny): build T_v (1024, 512) on device via PE matmul
          (with PE transposes for the lhsT/rhs layouts) + bn_stats LayerNorm.
  phase 2 (memory-bound): indirect-DMA row gather of T_v rows per batch
          element, then contiguous store to the output slab.
"""

import math
import os

import numpy as np

import concourse.bacc as bacc
import concourse.bass as bass
import concourse.mybir as mybir
import concourse.tile as tile
from concourse.bass_utils import run_bass_kernel_spmd
from concourse.masks import make_identity

# ---------------------------------------------------------------- constants
N_VARS = 32
N_CORES = 8
VPC = N_VARS // N_CORES  # vars per core = 4
B = 8192
D = 512
TR = 1024  # table rows kept on device (indices are < 1000 per spec)
EPS = 1e-5
CARDS = [1000, 5000, 20000, 50000] * 8
DIMS = [int(math.ceil(c**0.5) + 1) for c in CARDS]  # [33, 72, 143, 225] * 8
DIMS_LOCAL = DIMS[:VPC]  # same for every core

F32 = mybir.dt.float32
I16 = mybir.dt.int16

# gather chunking: 4 chunks of 2048 rows per variable
G_CHUNK_ROWS = 2048
G_CHUNKS = B // G_CHUNK_ROWS  # 4
G_FREE = G_CHUNK_ROWS // 128  # 16 rows per partition per chunk
G_SLOTS = G_CHUNK_ROWS // 16  # 128 int16 idx slots per 16-partition group


def _kchunks(d):
    """Split contraction dim d into <=128-partition chunks."""
    return [(k0, min(k0 + 128, d)) for k0 in range(0, d, 128)]


# ---------------------------------------------------------------- device IR
def _build_nc(repeat=1, phase="full", lean_ln=False, table_bf16=False):
    """repeat>1 duplicates the whole compute+gather body (same data) for
    dispatch-overhead-free wall-clock benchmarking via (T_k - T_1)/(k-1).
    phase: "full" | "pre" (no gather/store) | "pregather" (no output store),
    for component timing only (non-"full" variants produce wrong output).
    lean_ln=True compiles the specialization for proj_b==0, ln_g==1, ln_b==0
    (validated host-side before selecting this variant)."""
    nc = bacc.Bacc("TRN2", target_bir_lowering=False, debug=False)

    emb_in = [nc.dram_tensor(f"emb{v}", (TR, DIMS_LOCAL[v]), F32, kind="ExternalInput")
              for v in range(VPC)]
    w_in = [nc.dram_tensor(f"w{v}", (D, DIMS_LOCAL[v]), F32, kind="ExternalInput")
            for v in range(VPC)]
    # aux[v, 0] = proj_b, aux[v, 1] = ln_g, aux[v, 2] = ln_b
    aux_in = nc.dram_tensor("aux", (VPC, 3, D), F32, kind="ExternalInput")
    # dma_gather int16 indices: per (var, chunk), idx j -> partition j%16,
    # slot j//16, replicated across the 8 Q7-core 16-partition groups
    idx_in = nc.dram_tensor("idx", (128, VPC * G_CHUNKS * G_SLOTS), I16,
                            kind="ExternalInput")
    y_out = nc.dram_tensor("y", (VPC * B, D), F32, kind="ExternalOutput")

    # per-variable precomputed tables (device-internal).  bf16 halves the
    # gather's HBM read traffic; the store DMA casts back to f32 in-flight.
    t_dt = mybir.dt.bfloat16 if table_bf16 else F32
    t_dram = [nc.dram_tensor(f"t{v}", (TR, D), t_dt, kind="Internal")
              for v in range(VPC)]

    with tile.TileContext(nc) as tc:
        with (
            tc.tile_pool(name="consts", bufs=1) as consts,
            tc.tile_pool(name="wpool", bufs=2) as wpool,
            tc.tile_pool(name="lpool", bufs=3) as lpool,
            tc.tile_pool(name="tpose", bufs=2, space="PSUM") as tpose_psum,
            tc.tile_pool(name="mmps", bufs=2, space="PSUM") as mm_psum,
            tc.tile_pool(name="tpool", bufs=3) as tpool,
            tc.tile_pool(name="stat", bufs=4) as stat,
            tc.tile_pool(name="gpool", bufs=4) as gpool,
        ):
            identity = consts.tile([128, 128], F32)
            make_identity(nc, identity[:])
            eps_t = consts.tile([128, 1], F32)
            nc.vector.memset(eps_t[:], EPS)
            idx_sb = consts.tile([128, VPC * G_CHUNKS * G_SLOTS], I16)
            nc.sync.dma_start(out=idx_sb[:], in_=idx_in[:, :])

            for rep in range(repeat):
              for v in range(VPC):
                d = DIMS_LOCAL[v]
                kcs = _kchunks(d)

                # broadcast aux rows across partitions (DRE replication DMA)
                pb_b = g_b = bb_b = None
                if not lean_ln:
                    aux_t = []
                    for a in range(3):
                        t = wpool.tile([128, D], F32, tag=f"aux{a}",
                                       name=f"aux_v{v}_{a}")
                        src = aux_in[v, a, :]
                        bcast = bass.AP(tensor=src.tensor, offset=src.offset,
                                        ap=[[0, 128]] + src.ap)
                        nc.gpsimd.dma_start(out=t[:], in_=bcast)
                        aux_t.append(t)
                    pb_b, g_b, bb_b = aux_t

                # rhs = W^T  (d, 512) assembled via PE transposes of W tiles
                rhs = [wpool.tile([128, D], F32, tag=f"rhs{kci}",
                                  name=f"rhs_v{v}_{kci}")
                       for kci in range(len(kcs))]
                for t4 in range(D // 128):
                    w_t = wpool.tile([128, d], F32, tag="wstage")
                    nc.sync.dma_start(out=w_t[:],
                                      in_=w_in[v][t4 * 128:(t4 + 1) * 128, :])
                    for kci, (k0, k1) in enumerate(kcs):
                        kk = k1 - k0
                        tp = tpose_psum.tile([128, 128], F32)
                        nc.tensor.transpose(out=tp[:kk, :], in_=w_t[:, k0:k1],
                                            identity=identity[:])
                        nc.vector.tensor_copy(
                            out=rhs[kci][:kk, t4 * 128:(t4 + 1) * 128],
                            in_=tp[:kk, :])

                # per 128-row tile of the table: matmul + LayerNorm
                for s in range(TR // 128):
                    e_t = lpool.tile([128, d], F32, tag="emb")
                    nc.sync.dma_start(out=e_t[:],
                                      in_=emb_in[v][s * 128:(s + 1) * 128, :])
                    lts = []
                    for kci, (k0, k1) in enumerate(kcs):
                        kk = k1 - k0
                        tp = tpose_psum.tile([128, 128], F32)
                        nc.tensor.transpose(out=tp[:kk, :], in_=e_t[:, k0:k1],
                                            identity=identity[:])
                        lt = lpool.tile([128, 128], F32, tag=f"lhsT{kci}")
                        nc.vector.tensor_copy(out=lt[:kk, :], in_=tp[:kk, :])
                        lts.append(lt)

                    mm = mm_psum.tile([128, D], F32)
                    for kci, (k0, k1) in enumerate(kcs):
                        kk = k1 - k0
                        nc.tensor.matmul(
                            out=mm[:, :],
                            lhsT=lts[kci][:kk, :],
                            rhs=rhs[kci][:kk, :],
                            start=(kci == 0),
                            stop=(kci == len(kcs) - 1),
                        )

                    y_t = tpool.tile([128, D], F32, tag="yt")
                    stats = stat.tile([128, 6], F32, tag="bn")
                    mv = stat.tile([128, 2], F32, tag="mv")
                    if lean_ln:
                        # proj_b==0, g==1, beta==0: stats straight off PSUM,
                        # single normalize pass PSUM->SBUF
                        nc.vector.bn_stats(out=stats[:], in_=mm[:, :])
                    else:
                        # y = mm + proj_b  (reads PSUM, writes SBUF)
                        nc.vector.tensor_add(out=y_t[:], in0=mm[:, :],
                                             in1=pb_b[:])
                        nc.vector.bn_stats(out=stats[:], in_=y_t[:])
                    nc.vector.bn_aggr(out=mv[:], in_=stats[:])
                    # mv[:,1] <- 1/sqrt(var + eps)
                    nc.scalar.activation(
                        out=mv[:, 1:2], in_=mv[:, 1:2],
                        func=mybir.ActivationFunctionType.Sqrt,
                        bias=eps_t[:], scale=1.0, alpha=0.0)
                    nc.vector.reciprocal(out=mv[:, 1:2], in_=mv[:, 1:2])
                    # y = (y - mean) * rstd
                    nc.vector.tensor_scalar(
                        out=y_t[:], in0=mm[:, :] if lean_ln else y_t[:],
                        scalar1=mv[:, 0:1], scalar2=mv[:, 1:2],
                        op0=mybir.AluOpType.subtract,
                        op1=mybir.AluOpType.mult)
                    if not lean_ln:
                        # y = y * g + beta
                        nc.vector.tensor_mul(out=y_t[:], in0=y_t[:],
                                             in1=g_b[:])
                        nc.vector.tensor_add(out=y_t[:], in0=y_t[:],
                                             in1=bb_b[:])
                    if table_bf16:
                        # dtype cast during DMA is SWDGE-only
                        nc.gpsimd.dma_start(
                            out=t_dram[v][s * 128:(s + 1) * 128, :],
                            in_=y_t[:])
                    else:
                        nc.sync.dma_start(
                            out=t_dram[v][s * 128:(s + 1) * 128, :],
                            in_=y_t[:])

                # gather phase: T_v[x[b]] -> output slab rows [v*B, (v+1)*B)
                if phase == "pre":
                    continue
                for ci in range(G_CHUNKS):
                    gt = gpool.tile([128, G_FREE, D], t_dt, tag="gather")
                    off0 = (v * G_CHUNKS + ci) * G_SLOTS
                    nc.gpsimd.dma_gather(
                        out_ap=gt[:, :, :],
                        in_ap=t_dram[v][:, :],
                        idxs_ap=idx_sb[:, off0:off0 + G_SLOTS],
                        num_idxs=G_CHUNK_ROWS,
                        num_idxs_reg=G_CHUNK_ROWS,
                        elem_size=D,
                        single_packet=False,
                    )
                    if phase == "pregather":
                        continue
                    # gathered row j lands at (p=j%128, c=j//128)
                    row0 = v * B + ci * G_CHUNK_ROWS
                    y_view = y_out[row0:row0 + G_CHUNK_ROWS, :].rearrange(
                        "(c p) d -> p c d", p=128)
                    if table_bf16:
                        # bf16 -> f32 cast during the store (SWDGE-only)
                        nc.gpsimd.dma_start(out=y_view, in_=gt[:, :, :])
                    else:
                        # alternate the two HWDGE rings (SP / ACT)
                        eng = nc.sync if ci % 2 == 0 else nc.scalar
                        eng.dma_start(out=y_view, in_=gt[:, :, :])

    nc.compile()
    return nc


_NC_CACHE = {}


def _get_nc(lean_ln=False):
    key = ("nc", lean_ln)
    if key not in _NC_CACHE:
        _NC_CACHE[key] = _build_nc(lean_ln=lean_ln)
    return _NC_CACHE[key]


# ---------------------------------------------------------------- host side
def _numpy_fallback(emb_tables, proj_w, proj_b, ln_g, ln_b, x):
    outs = []
    for i in range(len(emb_tables)):
        e = np.asarray(emb_tables[i], np.float32)[x[:, i]]
        y = e @ np.asarray(proj_w[i], np.float32).T + np.asarray(
            proj_b[i], np.float32)
        mu = y.mean(-1, keepdims=True, dtype=np.float32)
        var = np.square(y - mu).mean(-1, keepdims=True, dtype=np.float32)
        y = (y - mu) / np.sqrt(var + EPS)
        outs.append(y * np.asarray(ln_g[i], np.float32)
                    + np.asarray(ln_b[i], np.float32))
    return np.stack(outs).astype(np.float32)


def _prep_in_maps(emb_tables, proj_w, proj_b, ln_g, ln_b, x):
    in_maps = []
    for c in range(N_CORES):
        m = {}
        for v in range(VPC):
            gv = c * VPC + v
            d = DIMS[gv]
            et = np.asarray(emb_tables[gv], np.float32)
            if et.shape[0] >= TR:
                e = np.ascontiguousarray(et[:TR])
            else:
                e = np.zeros((TR, d), np.float32)
                e[:et.shape[0]] = et
            m[f"emb{v}"] = e
            m[f"w{v}"] = np.ascontiguousarray(np.asarray(proj_w[gv], np.float32))
        aux = np.empty((VPC, 3, D), np.float32)
        for v in range(VPC):
            gv = c * VPC + v
            aux[v, 0] = np.asarray(proj_b[gv], np.float32)
            aux[v, 1] = np.asarray(ln_g[gv], np.float32)
            aux[v, 2] = np.asarray(ln_b[gv], np.float32)
        m["aux"] = aux
        idx = np.empty((16, VPC, G_CHUNKS, G_SLOTS), np.int16)
        for v in range(VPC):
            gv = c * VPC + v
            xv = x[:, gv].astype(np.int16)
            # chunk ci, idx j -> partition j%16, slot j//16
            idx[:, v, :, :] = xv.reshape(G_CHUNKS, G_SLOTS, 16).transpose(
                2, 0, 1)
        idx = np.tile(idx.reshape(16, VPC * G_CHUNKS * G_SLOTS), (8, 1))
        m["idx"] = np.ascontiguousarray(idx)
        in_maps.append(m)
    return in_maps


def _run(emb_tables, proj_w, proj_b, ln_g, ln_b, x, trace=False):
    """Returns (out, exec_time_ns or None)."""
    x = np.asarray(x)
    ok = (
        len(emb_tables) == N_VARS
        and x.shape == (B, N_VARS)
        and all(np.asarray(emb_tables[i]).shape == (CARDS[i], DIMS[i])
                for i in range(N_VARS))
        and all(np.asarray(proj_w[i]).shape == (D, DIMS[i])
                for i in range(N_VARS))
        and int(x.min()) >= 0
        and int(x.max()) < TR
    )
    if not ok:
        return _numpy_fallback(emb_tables, proj_w, proj_b, ln_g, ln_b, x), None

    in_maps = _prep_in_maps(emb_tables, proj_w, proj_b, ln_g, ln_b, x)
    lean = (not np.any(np.asarray(proj_b))
            and np.all(np.asarray(ln_g) == 1.0)
            and not np.any(np.asarray(ln_b)))
    nc = _get_nc(lean_ln=lean)
    kw = {}
    if trace:
        kw = dict(trace=True, trace_cores=list(range(N_CORES)))
    res = run_bass_kernel_spmd(nc, in_maps, core_ids=list(range(N_CORES)), **kw)
    out = np.concatenate(
        [r["y"].reshape(VPC, B, D) for r in res.results], axis=0)
    return out, res.exec_time_ns


def kernel(emb_tables, proj_w, proj_b, ln_g, ln_b, x):
    out, _ = _run(emb_tables, proj_w, proj_b, ln_g, ln_b, x,
                  trace=bool(int(os.environ.get("KERNEL_TRACE", "0"))))
    return out
